# revision 2
# baseline (speedup 1.0000x reference)
"""Chamfer loss kernel for Trainium2, 8 NeuronCores.

Strategy (data-parallel over B):
  - B=16 batches sharded 2-per-core across 8 cores.
  - Per batch, compute S = -d^2 = 2*k1.k2 - |k1|^2 - |k2|^2 directly on the
    tensor engine via an augmented K=13 matmul in float16 using an exact
    hi/lo split (Dekker-style):  x = hi(x) + lo(x), products computed in
    f16 pairs (hh, hl, lh) and accumulated in fp32 PSUM.  This runs at
    1 cycle/row on the PE (vs 4 for native fp32) with ~f32 precision.
  - Pass F tiles S as [m(128), n(512)] -> row max over n = -min_n d^2
    (forward) plus argmax via max_index -> forward argmin indices.
  - Pass B tiles S^T as [n(128), m(512)] -> backward mins/argmins.
  - Host gathers sigma at the argmin indices and does the (tiny) final
    loss math exactly as the reference does.

The two passes use identical K-accumulation order (products bitwise equal,
scaling by 2 is exact) so S_F and S_B^T round identically.
"""

import numpy as np

import concourse.bacc as bacc
import concourse.mybir as mybir
from concourse.tile import TileContext
from concourse.bass_utils import run_bass_kernel_spmd

B, D3, M, N = 16, 3, 4096, 4096
NCORES = 8
BL = B // NCORES  # batches per core = 2
EPS = 1e-12
KAUG = 13  # 3x3 f16-split cross groups + 4 aug rows

MT = M // 128  # 32 m-tiles per batch
NT = N // 512  # 8  n-tiles of 512
F32 = mybir.dt.float32
F16 = mybir.dt.float16
U32 = mybir.dt.uint32

_CACHED_NC = None


def _halving_rowmax(nc, pool, rowbuf, out_col):
    """Max over the free dim of rowbuf [128, 4096] -> out_col [128, 1]."""
    h1 = pool.tile([128, 2048], F32, tag="h1")
    nc.vector.tensor_tensor(
        out=h1, in0=rowbuf[:, 0:2048], in1=rowbuf[:, 2048:4096],
        op=mybir.AluOpType.max,
    )
    h2 = pool.tile([128, 1024], F32, tag="h2")
    nc.vector.tensor_tensor(
        out=h2, in0=h1[:, 0:1024], in1=h1[:, 1024:2048],
        op=mybir.AluOpType.max,
    )
    h3 = pool.tile([128, 512], F32, tag="h3")
    nc.vector.tensor_tensor(
        out=h3, in0=h2[:, 0:512], in1=h2[:, 512:1024],
        op=mybir.AluOpType.max,
    )
    nc.vector.tensor_reduce(
        out=out_col, in_=h3, axis=mybir.AxisListType.X, op=mybir.AluOpType.max,
    )


def build_nc():
    nc = bacc.Bacc(None, target_bir_lowering=False)

    # Per-core inputs: pre-augmented f16-split matmul operands, [BL, 13, M].
    afwd = nc.dram_tensor("afwd", [BL, KAUG, M], F16, kind="ExternalInput")
    bfwd = nc.dram_tensor("bfwd", [BL, KAUG, N], F16, kind="ExternalInput")
    abwd = nc.dram_tensor("abwd", [BL, KAUG, N], F16, kind="ExternalInput")
    bbwd = nc.dram_tensor("bbwd", [BL, KAUG, M], F16, kind="ExternalInput")

    # Outputs: row maxima of S (= -min d^2) and argmax indices, both
    # laid out [128, 32] with element (p, t) = row t*128+p.
    smaxf = nc.dram_tensor("smaxf", [BL, 128, MT], F32, kind="ExternalOutput")
    smaxb = nc.dram_tensor("smaxb", [BL, 128, MT], F32, kind="ExternalOutput")
    idxf = nc.dram_tensor("idxf", [BL, 128, MT], U32, kind="ExternalOutput")
    idxb = nc.dram_tensor("idxb", [BL, 128, MT], U32, kind="ExternalOutput")

    with TileContext(nc) as tc:
        with (
            tc.tile_pool(name="aug", bufs=1) as aug_pool,
            tc.tile_pool(name="rows", bufs=3) as row_pool,
            tc.tile_pool(name="halv", bufs=2) as halv_pool,
            tc.tile_pool(name="smax", bufs=2) as smax_pool,
            tc.tile_pool(name="idx", bufs=2) as idx_pool,
            tc.tile_pool(name="ps", bufs=4, space="PSUM") as psum_pool,
        ):
            for b in range(BL):
                # Load augmented operands for this batch.
                a_f = aug_pool.tile([KAUG, M], F16, tag="a_f")
                b_f = aug_pool.tile([KAUG, N], F16, tag="b_f")
                a_b = aug_pool.tile([KAUG, N], F16, tag="a_b")
                b_b = aug_pool.tile([KAUG, M], F16, tag="b_b")
                nc.sync.dma_start(out=a_f[:], in_=afwd[b])
                nc.sync.dma_start(out=b_f[:], in_=bfwd[b])
                nc.sync.dma_start(out=a_b[:], in_=abwd[b])
                nc.sync.dma_start(out=b_b[:], in_=bbwd[b])

                # Two symmetric passes: (lhsT, rhs, outputs)
                for lhsT, rhs, smax_dram, idx_dram in (
                    (a_f, b_f, smaxf, idxf),
                    (a_b, b_b, smaxb, idxb),
                ):
                    smax_sb = smax_pool.tile([128, MT], F32, tag="smax_sb")
                    idx_sb = idx_pool.tile([128, MT], U32, tag="idx_sb")
                    for mt in range(MT):
                        rowbuf = row_pool.tile([128, 4096], F32, tag="rowbuf")
                        for nt in range(NT):
                            ps = psum_pool.tile([128, 512], F32, tag="s")
                            nc.tensor.matmul(
                                ps,
                                lhsT[:, mt * 128:(mt + 1) * 128],
                                rhs[:, nt * 512:(nt + 1) * 512],
                                start=True, stop=True,
                            )
                            nc.scalar.copy(
                                out=rowbuf[:, nt * 512:(nt + 1) * 512], in_=ps[:]
                            )
                        _halving_rowmax(
                            nc, halv_pool, rowbuf, smax_sb[:, mt:mt + 1]
                        )
                        idx8 = idx_pool.tile([128, 8], U32, tag="idx8")
                        nc.vector.max_index(
                            out=idx8,
                            in_max=smax_sb[:, mt:mt + 1].to_broadcast([128, 8]),
                            in_values=rowbuf,
                        )
                        nc.vector.tensor_copy(
                            idx_sb[:, mt:mt + 1], idx8[:, 0:1]
                        )
                    nc.sync.dma_start(out=smax_dram[b], in_=smax_sb[:])
                    nc.sync.dma_start(out=idx_dram[b], in_=idx_sb[:])
    nc.compile()
    return nc


def _get_nc():
    global _CACHED_NC
    if _CACHED_NC is None:
        _CACHED_NC = build_nc()
    return _CACHED_NC


def _split16(x):
    h = x.astype(np.float16)
    l = (x - h.astype(np.float32)).astype(np.float16)
    return h, l


def _prep_core_inputs(k1, k2):
    """k1, k2: [BL, 3, 4096] float32 -> augmented f16-split operand dict.

    Pair groups (lhsT_row, rhs_row), accumulated in this order:
      F: (ah,bh)x3 (ah,bl)x3 (al,bh)x3 (s1h,-1) (s1l,-1) (1,-s2h) (1,-s2l)
      Bw:(bh,ah)x3 (bl,ah)x3 (bh,al)x3 (1,-s1h) (1,-s1l) (s2h,-1) (s2l,-1)
    With a = 2*k1 and b = k2, each Bw summand is bitwise equal to the F
    summand at the same position (x2 scaling is exact in f16), so
    S_F == S_B^T bitwise.
    """
    sq1 = np.sum(k1 * k1, axis=1)  # [BL, M] f32
    sq2 = np.sum(k2 * k2, axis=1)  # [BL, N] f32
    onesM = np.ones_like(sq1)[:, None, :]
    onesN = np.ones_like(sq2)[:, None, :]

    ah, al = _split16(2.0 * k1)        # [BL, 3, M]
    bh, bl = _split16(k2)              # [BL, 3, N]
    s1h, s1l = _split16(sq1)           # [BL, M]
    s2h, s2l = _split16(sq2)           # [BL, N]
    s1h, s1l = s1h[:, None, :], s1l[:, None, :]
    s2h, s2l = s2h[:, None, :], s2l[:, None, :]

    f16 = np.float16
    afwd = np.concatenate(
        [ah, ah, al, s1h.astype(f16), s1l.astype(f16),
         onesM.astype(f16), onesM.astype(f16)], axis=1
    ).astype(f16)
    bfwd = np.concatenate(
        [bh, bl, bh, -onesN.astype(f16), -onesN.astype(f16),
         -s2h.astype(f16), -s2l.astype(f16)], axis=1
    ).astype(f16)

    # Backward pass: lhsT carries the k2 side scaled by 2, rhs the k1 side.
    abh, abl = _split16(2.0 * k2)
    bbh, bbl = _split16(k1)
    abwd = np.concatenate(
        [abh, abl, abh, onesN.astype(f16), onesN.astype(f16),
         s2h.astype(f16), s2l.astype(f16)], axis=1
    ).astype(f16)
    bbwd = np.concatenate(
        [bbh, bbh, bbl, -s1h.astype(f16), -s1l.astype(f16),
         -onesM.astype(f16), -onesM.astype(f16)], axis=1
    ).astype(f16)
    return {"afwd": afwd, "bfwd": bfwd, "abwd": abwd, "bbwd": bbwd}


def run_device(keypoints1, keypoints2, trace=False):
    """Returns per-core result dicts."""
    nc = _get_nc()
    in_maps = []
    for c in range(NCORES):
        sl = slice(c * BL, (c + 1) * BL)
        in_maps.append(_prep_core_inputs(
            np.asarray(keypoints1[sl]), np.asarray(keypoints2[sl])
        ))
    res = run_bass_kernel_spmd(
        nc, in_maps, list(range(NCORES)), trace=trace
    )
    return res


def _finish_host(results, sigma1, sigma2):
    """Combine per-core device outputs into the scalar loss (float32 math)."""
    sigma1 = np.asarray(sigma1)
    sigma2 = np.asarray(sigma2)
    fwd_terms = np.zeros((B, M), np.float32)
    bwd_terms = np.zeros((B, N), np.float32)
    for c in range(NCORES):
        r = results[c]
        for bl in range(BL):
            bg = c * BL + bl
            # [128, 32] -> [4096] with m = t*128 + p
            neg_d2_f = -r["smaxf"][bl].T.reshape(M)
            neg_d2_b = -r["smaxb"][bl].T.reshape(N)
            ixf = r["idxf"][bl].T.reshape(M).astype(np.int64)
            ixb = r["idxb"][bl].T.reshape(N).astype(np.int64)
            min_f = np.sqrt(np.maximum(neg_d2_f, EPS).astype(np.float32))
            min_b = np.sqrt(np.maximum(neg_d2_b, EPS).astype(np.float32))
            sig_f = (sigma1[bg] + sigma2[bg][ixf]) * np.float32(0.5)
            sig_b = (sigma2[bg] + sigma1[bg][ixb]) * np.float32(0.5)
            fwd_terms[bg] = np.log(sig_f) + min_f / sig_f
            bwd_terms[bg] = np.log(sig_b) + min_b / sig_b
    loss = fwd_terms.mean(dtype=np.float32) + bwd_terms.mean(dtype=np.float32)
    return np.float32(loss)


def kernel(keypoints1, keypoints2, sigma1, sigma2):
    res = run_device(keypoints1, keypoints2)
    return _finish_host(res.results, sigma1, sigma2)


# revision 20
# speedup vs baseline: 96.2997x; 96.2997x over previous
"""Chamfer loss kernel for Trainium2, 8 NeuronCores.

Strategy (data-parallel over B, 2 batches/core):
  - S = -d^2 = 2*k1.k2 - |k1|^2 - |k2|^2 computed on the tensor engine via an
    augmented K=13 matmul in float16 with an exact hi/lo (Dekker) split:
    f32-class precision at 1 cycle/row.
  - Two symmetric passes per batch: pass F tiles S as [m(128), n(512)]
    (forward rows), pass B tiles S^T as [n(128), m(512)] (backward rows).
  - Per 128-row tile: DVE tensor_scalar with op1=max accumulator fuses the
    PSUM->SBUF copy with per-chunk row maxima; a tiny reduce gives the row
    max (= -min d^2, exact).
  - sigma selection without argmin indices: a {0,1} mask marking row-max
    positions is built against the SAME pass's row max (exact compare;
    alternating ACT Sign anti-mask / GPSIMD tensor_scalar is_ge positive
    mask by row-tile parity to balance engines), DMA-transposed (batched
    32-block transpose), then PE-contracted with [sigma_hi, sigma_lo, 1]
    to produce [sum sigma at max, count] per opposite-side row.
  - Host: sigma_sel = (hi+lo)/cnt (positive rows) or complement via the
    total sigma sums (anti rows), then the tiny final loss math.
"""

import numpy as np
import ml_dtypes

import concourse.bacc as bacc
import concourse.mybir as mybir
from concourse.tile import TileContext
from concourse.bass_utils import run_bass_kernel_spmd

B, D3, M, N = 16, 3, 4096, 4096
NCORES = 8
BL = B // NCORES  # batches per core = 2
EPS = 1e-12
KAUG = 13

MT = M // 128  # 32 row-tiles per pass
NT = N // 512  # 8 column chunks of 512
F32 = mybir.dt.float32
F16 = mybir.dt.float16
BF16 = mybir.dt.bfloat16

_CACHED_NC = None


def build_nc():
    nc = bacc.Bacc(None, target_bir_lowering=False)

    afwd = nc.dram_tensor("afwd", [BL, KAUG, M], F16, kind="ExternalInput")
    bfwd = nc.dram_tensor("bfwd", [BL, KAUG, N], F16, kind="ExternalInput")
    abwd = nc.dram_tensor("abwd", [BL, KAUG, N], F16, kind="ExternalInput")
    bbwd = nc.dram_tensor("bbwd", [BL, KAUG, M], F16, kind="ExternalInput")
    s1ones = nc.dram_tensor("s1ones", [BL, 128, 3 * MT], BF16, kind="ExternalInput")
    s2ones = nc.dram_tensor("s2ones", [BL, 128, 3 * MT], BF16, kind="ExternalInput")

    smaxf = nc.dram_tensor("smaxf", [BL, 128, MT], F32, kind="ExternalOutput")
    smaxb = nc.dram_tensor("smaxb", [BL, 128, MT], F32, kind="ExternalOutput")
    usef = nc.dram_tensor("self_", [BL, 128, 3 * MT], F32, kind="ExternalOutput")
    useb = nc.dram_tensor("selb_", [BL, 128, 3 * MT], F32, kind="ExternalOutput")

    with TileContext(nc) as tc:
        with (
            tc.tile_pool(name="aug", bufs=1) as aug_pool,
            tc.tile_pool(name="rows", bufs=3) as row_pool,
            tc.tile_pool(name="rmax", bufs=2) as rmax_pool,
            tc.tile_pool(name="smax", bufs=2) as smax_pool,
            tc.tile_pool(name="mask", bufs=3) as mask_pool,
            tc.tile_pool(name="sel", bufs=2) as sel_pool,
            tc.tile_pool(name="ps", bufs=3, space="PSUM") as psum_pool,
            tc.tile_pool(name="psel", bufs=1, space="PSUM") as psel_pool,
        ):
            for b in range(BL):
                a_f = aug_pool.tile([KAUG, M], F16, tag="a_f")
                b_f = aug_pool.tile([KAUG, N], F16, tag="b_f")
                a_b = aug_pool.tile([KAUG, N], F16, tag="a_b")
                b_b = aug_pool.tile([KAUG, M], F16, tag="b_b")
                s1o = aug_pool.tile([128, 3 * MT], BF16, tag="s1o")
                s2o = aug_pool.tile([128, 3 * MT], BF16, tag="s2o")
                nc.sync.dma_start(out=a_f[:], in_=afwd[b])
                nc.sync.dma_start(out=b_f[:], in_=bfwd[b])
                nc.sync.dma_start(out=a_b[:], in_=abwd[b])
                nc.sync.dma_start(out=b_b[:], in_=bbwd[b])
                nc.sync.dma_start(out=s1o[:], in_=s1ones[b])
                nc.sync.dma_start(out=s2o[:], in_=s2ones[b])

                # (lhsT, rhs, sigma-of-other-side, smax out, sel out)
                for lhsT, rhs, sother, smax_dram, sel_dram in (
                    (a_f, b_f, s2o, smaxf, usef),
                    (a_b, b_b, s1o, smaxb, useb),
                ):
                    smax_sb = smax_pool.tile([128, MT], F32, tag="smax_sb")
                    psel = psel_pool.tile([128, 3 * MT], F32, tag="psel")

                    def emit_contracts(rt, maskt, psel=psel, sother=sother):
                        for cc in range(MT):
                            nc.tensor.matmul(
                                psel[:, rt * 3:(rt + 1) * 3],
                                maskt[:, cc, :],
                                sother[:, cc * 3:(cc + 1) * 3],
                                start=(cc == 0), stop=(cc == MT - 1),
                            )

                    pending = None
                    for rt in range(MT):  # row-tiles of this pass
                        rowbuf = row_pool.tile([128, 4096], F32, tag="rowbuf")
                        rmax8 = rmax_pool.tile([128, NT // 2], F32, tag="rmax8")
                        for ch in range(NT // 2):
                            ps = psum_pool.tile([128, 1024], F32, tag="s")
                            for half in range(2):
                                nc.tensor.matmul(
                                    ps[:, half * 512:(half + 1) * 512],
                                    lhsT[:, rt * 128:(rt + 1) * 128],
                                    rhs[:, (2 * ch + half) * 512:
                                        (2 * ch + half + 1) * 512],
                                    start=True, stop=True,
                                )
                            # fused copy + chunk row-max on DVE
                            nc.vector.tensor_scalar(
                                out=rowbuf[:, ch * 1024:(ch + 1) * 1024],
                                in0=ps[:], scalar1=0.0, scalar2=None,
                                op0=mybir.AluOpType.add,
                                op1=mybir.AluOpType.max,
                                accum_out=rmax8[:, ch:ch + 1],
                            )
                        nc.vector.tensor_reduce(
                            out=smax_sb[:, rt:rt + 1], in_=rmax8,
                            axis=mybir.AxisListType.X, op=mybir.AluOpType.max,
                        )
                        # mask of row-max positions (exact same-pass compare);
                        # parity alternation balances ACT vs GPSIMD.
                        mask = mask_pool.tile([128, 4096], BF16, tag="mask")
                        if rt % 2 == 0:
                            # anti-mask: {0 at max, 1 elsewhere}
                            nc.scalar.activation(
                                out=mask[:], in_=rowbuf[:],
                                func=mybir.ActivationFunctionType.Sign,
                                bias=smax_sb[:, rt:rt + 1], scale=-1.0,
                            )
                        else:
                            # positive mask: {1 at max, 0 elsewhere}
                            nc.gpsimd.tensor_scalar(
                                out=mask[:], in0=rowbuf[:],
                                scalar1=smax_sb[:, rt:rt + 1], scalar2=None,
                                op0=mybir.AluOpType.is_ge,
                            )
                        maskt = mask_pool.tile([128, MT, 128], BF16, tag="maskt")
                        nc.sync.dma_start_transpose(out=maskt[:], in_=mask[:])
                        if pending is not None:
                            emit_contracts(*pending)
                        pending = (rt, maskt)
                    emit_contracts(*pending)

                    nc.sync.dma_start(out=smax_dram[b], in_=smax_sb[:])
                    sel_sb = sel_pool.tile([128, 3 * MT], F32, tag="sel_sb")
                    nc.scalar.copy(out=sel_sb[:], in_=psel[:])
                    nc.sync.dma_start(out=sel_dram[b], in_=sel_sb[:])
    nc.compile()
    return nc


def _get_nc():
    global _CACHED_NC
    if _CACHED_NC is None:
        _CACHED_NC = build_nc()
    return _CACHED_NC


def _split16(x):
    h = x.astype(np.float16)
    l = (x - h.astype(np.float32)).astype(np.float16)
    return h, l


def _splitbf(x):
    h = x.astype(ml_dtypes.bfloat16)
    l = (x - h.astype(np.float32)).astype(ml_dtypes.bfloat16)
    return h, l


def _prep_core_inputs(k1, k2, sig1, sig2):
    """k1,k2: [BL,3,4096] f32; sig1,sig2: [BL,4096] f32."""
    sq1 = np.sum(k1 * k1, axis=1)
    sq2 = np.sum(k2 * k2, axis=1)
    onesM = np.ones_like(sq1)[:, None, :]
    onesN = np.ones_like(sq2)[:, None, :]

    ah, al = _split16(2.0 * k1)
    bh, bl = _split16(k2)
    s1h, s1l = _split16(sq1)
    s2h, s2l = _split16(sq2)
    s1h, s1l = s1h[:, None, :], s1l[:, None, :]
    s2h, s2l = s2h[:, None, :], s2l[:, None, :]

    f16 = np.float16
    afwd = np.concatenate(
        [ah, ah, al, s1h.astype(f16), s1l.astype(f16),
         onesM.astype(f16), onesM.astype(f16)], axis=1).astype(f16)
    bfwd = np.concatenate(
        [bh, bl, bh, -onesN.astype(f16), -onesN.astype(f16),
         -s2h.astype(f16), -s2l.astype(f16)], axis=1).astype(f16)
    abh, abl = _split16(2.0 * k2)
    bbh, bbl = _split16(k1)
    abwd = np.concatenate(
        [abh, abl, abh, onesN.astype(f16), onesN.astype(f16),
         s2h.astype(f16), s2l.astype(f16)], axis=1).astype(f16)
    bbwd = np.concatenate(
        [bbh, bbh, bbl, -s1h.astype(f16), -s1l.astype(f16),
         -onesM.astype(f16), -onesM.astype(f16)], axis=1).astype(f16)

    def sig_layout(sig):
        hi, lo = _splitbf(sig)
        out = np.zeros((sig.shape[0], 128, 3 * MT), ml_dtypes.bfloat16)
        hi_r = hi.reshape(-1, MT, 128)
        lo_r = lo.reshape(-1, MT, 128)
        out[:, :, 0::3] = np.transpose(hi_r, (0, 2, 1))
        out[:, :, 1::3] = np.transpose(lo_r, (0, 2, 1))
        out[:, :, 2::3] = 1.0
        return out

    return {"afwd": afwd, "bfwd": bfwd, "abwd": abwd, "bbwd": bbwd,
            "s1ones": sig_layout(sig1), "s2ones": sig_layout(sig2)}


def run_device(keypoints1, keypoints2, sigma1, sigma2, trace=False):
    nc = _get_nc()
    in_maps = []
    for c in range(NCORES):
        sl = slice(c * BL, (c + 1) * BL)
        in_maps.append(_prep_core_inputs(
            np.asarray(keypoints1[sl]), np.asarray(keypoints2[sl]),
            np.asarray(sigma1[sl]), np.asarray(sigma2[sl]),
        ))
    return run_bass_kernel_spmd(nc, in_maps, list(range(NCORES)), trace=trace)


def _decode_sel(sel, sig_other):
    """sel: [128, 3*MT] device sums; sig_other: [4096] f32 of the
    contracted sigma.  Returns sigma at the row max per row (4096)."""
    s = sel.reshape(128, MT, 3).transpose(1, 0, 2).reshape(-1, 3)
    rt_idx = np.repeat(np.arange(MT), 128)
    even = (rt_idx % 2 == 0)  # anti rows -> complement
    hi = sig_other.astype(ml_dtypes.bfloat16).astype(np.float32)
    lo = (sig_other - hi).astype(ml_dtypes.bfloat16).astype(np.float32)
    tot_hi = np.float32(hi.astype(np.float64).sum())
    tot_lo = np.float32(lo.astype(np.float64).sum())
    cnt = np.where(even, np.float32(len(sig_other)) - s[:, 2], s[:, 2])
    cnt = np.maximum(cnt, 1.0)
    val = np.where(even, (tot_hi - s[:, 0]) + (tot_lo - s[:, 1]),
                   s[:, 0] + s[:, 1])
    return (val / cnt).astype(np.float32)


def _finish_host(results, sigma1, sigma2):
    sigma1 = np.asarray(sigma1)
    sigma2 = np.asarray(sigma2)
    fwd_terms = np.zeros((B, M), np.float32)
    bwd_terms = np.zeros((B, N), np.float32)
    for c in range(NCORES):
        r = results[c]
        for bl in range(BL):
            bg = c * BL + bl
            neg_d2_f = -r["smaxf"][bl].T.reshape(M)
            neg_d2_b = -r["smaxb"][bl].T.reshape(N)
            min_f = np.sqrt(np.maximum(neg_d2_f, EPS).astype(np.float32))
            min_b = np.sqrt(np.maximum(neg_d2_b, EPS).astype(np.float32))
            sig2sel = _decode_sel(r["self_"][bl], sigma2[bg])
            sig1sel = _decode_sel(r["selb_"][bl], sigma1[bg])
            sig_f = (sigma1[bg] + sig2sel) * np.float32(0.5)
            sig_b = (sigma2[bg] + sig1sel) * np.float32(0.5)
            fwd_terms[bg] = np.log(sig_f) + min_f / sig_f
            bwd_terms[bg] = np.log(sig_b) + min_b / sig_b
    loss = fwd_terms.mean(dtype=np.float32) + bwd_terms.mean(dtype=np.float32)
    return np.float32(loss)


def kernel(keypoints1, keypoints2, sigma1, sigma2):
    res = run_device(keypoints1, keypoints2, sigma1, sigma2)
    return _finish_host(res.results, sigma1, sigma2)
